# revision 4
# baseline (speedup 1.0000x reference)
"""HFreqC layer kernel for 8 Trainium2 NeuronCores.

The reference op (FFT -> zero centered low-freq band -> IFFT -> real -> relu)
is, up to the relu, a fixed real linear operator along the channel axis:
    y = relu(x @ W),  W = Re(ifft(mask * fft(I)))^T   (728x728, symmetric)

Strategy: pure data parallel over rows. 32*38*38 = 46208 rows = 361 tiles of
128; cores 0-6 take 45 tiles (+1 zero pad tile), core 7 takes 46, so every
core runs 46 tiles (5888 rows) and the critical path carries only 2% pad.
The host shards rows, lays each shard out channel-major (transposed tiles)
and casts to bf16; outputs return as bf16 and are upcast on the host, so
HBM traffic is halved in both directions. Per core:
  - W (row-padded to 768, bf16) lives in SBUF as 6 k-tiles [128, 728].
  - Tiles stream in groups of 8 (one contiguous [128, 8*768] bf16 DMA).
  - bf16 matmuls accumulate over 6 k-tiles into fp32 PSUM; each row-tile
    uses one 2-bank PSUM tile (j chunks at cols 0:364 and 512:876).
  - One fused relu+cast per row-tile on the PSUM->SBUF copy, alternating
    ScalarE / VectorE; contiguous bf16 DMA out.
"""

import numpy as np

C = 728            # channels
KT = 6             # k tiles of 128 (channel pad to 768)
CP = KT * 128      # 768 padded channels
TILES = 46         # 128-row tiles per core (45 real + 1 pad on cores 0-6)
N_CORES = 8
ROWS_TOTAL = 32 * 38 * 38          # 46208 = 361 * 128
JC = 364           # j-chunk width (2 chunks of 364; both >=256 for full rate)
GROUP_SIZES = [8] * 5 + [6]        # 46 tiles = 5 groups of 8 + 1 of 6
GMAX = 8

# per-core row ranges: cores 0-6 own 45 tiles, core 7 owns 46
_TILE_START = [i * 45 for i in range(N_CORES)]
_TILE_CNT = [45] * 7 + [46]

_CACHE = {}


def _build_w(scale: int) -> np.ndarray:
    """[CP, C] f32: W padded with zero rows; y_row = x_row @ W."""
    m_sh = np.ones(C)
    m_sh[C // 2 - C // scale: C // 2 + C // scale] = 0
    m = np.fft.ifftshift(m_sh)
    A = np.fft.ifft(m[:, None] * np.fft.fft(np.eye(C), axis=0), axis=0)
    W = np.real(A).T.astype(np.float32)
    Wp = np.zeros((CP, C), dtype=np.float32)
    Wp[:C] = W
    return Wp


def _bf16(a: np.ndarray):
    import ml_dtypes
    return a.astype(ml_dtypes.bfloat16)


def _shard_xt(xf: np.ndarray, core: int) -> np.ndarray:
    """[128, TILES*CP] bf16: [p][t*CP + u*128 + m] = x[(t0+t)*128+m, u*128+p]."""
    t0, cnt = _TILE_START[core], _TILE_CNT[core]
    xp = np.zeros((TILES * 128, CP), dtype=np.float32)
    xp[:cnt * 128, :C] = xf[t0 * 128: (t0 + cnt) * 128]
    v = _bf16(xp).reshape(TILES, 128, KT, 128)     # t m u p
    v = v.transpose(3, 0, 2, 1)                    # p t u m
    return np.ascontiguousarray(v).reshape(128, TILES * CP)


def _build_nc(repeat: int = 1, loop: int = 0):
    """loop>1 wraps the tile sweep in a hardware For_i loop executing it
    `loop` times per dispatch — used only for steady-state timing (the
    dispatch floor under axon is ~3-10ms, far above one sweep)."""
    import concourse.mybir as mybir
    import concourse.tile as tile
    from concourse import bacc

    fp32 = mybir.dt.float32
    bf16 = mybir.dt.bfloat16

    nc = bacc.Bacc("TRN2", target_bir_lowering=False)
    x_d = nc.dram_tensor("x", [128, TILES * CP], bf16, kind="ExternalInput").ap()
    w_d = nc.dram_tensor("w", [CP, C], bf16, kind="ExternalInput").ap()
    y_d = nc.dram_tensor("y", [128, TILES * C], bf16, kind="ExternalOutput").ap()

    w_v = w_d.rearrange("(u p) j -> p u j", u=KT, p=128)

    with tile.TileContext(nc) as tc:
        with (
            tc.tile_pool(name="wpool", bufs=1) as wpool,
            tc.tile_pool(name="io", bufs=4) as io,
            tc.tile_pool(name="psp", bufs=4, space="PSUM") as psp,
        ):
            w_sb = wpool.tile([128, KT * C], bf16)
            nc.sync.dma_start(out=w_sb.rearrange("p (u j) -> p u j", u=KT, j=C), in_=w_v)

            def sweep():
                for _r in range(repeat):
                    t0 = 0
                    tix = 0
                    for gs in GROUP_SIZES:
                        xt = io.tile([128, GMAX * CP], bf16, tag="xt")
                        nc.sync.dma_start(
                            out=xt[:, :gs * CP],
                            in_=x_d[:, t0 * CP:(t0 + gs) * CP])
                        ysb = io.tile([128, GMAX * C], bf16, tag="y")
                        for g in range(gs):
                            ps = psp.tile([128, 1024], fp32, tag="ps")
                            for jc in range(2):
                                for u in range(KT):
                                    nc.tensor.matmul(
                                        ps[:, jc * 512: jc * 512 + JC],
                                        lhsT=xt[:, g * CP + u * 128: g * CP + (u + 1) * 128],
                                        rhs=w_sb[:, u * C + jc * JC: u * C + (jc + 1) * JC],
                                        start=(u == 0),
                                        stop=(u == KT - 1),
                                    )
                            ps_v = ps.rearrange("p (c n) -> p c n", c=2, n=512)[:, :, :JC]
                            y_v = ysb[:, g * C:(g + 1) * C].rearrange(
                                "p (c n) -> p c n", c=2, n=JC)
                            if tix % 2 == 0:
                                nc.scalar.activation(
                                    y_v, ps_v, mybir.ActivationFunctionType.Relu)
                            else:
                                nc.vector.tensor_scalar_max(y_v, ps_v, 0.0)
                            tix += 1
                        nc.scalar.dma_start(
                            out=y_d[:, t0 * C:(t0 + gs) * C],
                            in_=ysb[:, :gs * C])
                        t0 += gs

            if loop > 1:
                hint = (mybir.EngineType.PE, mybir.EngineType.Activation,
                        mybir.EngineType.SP, mybir.EngineType.DVE)
                with tc.For_i(0, loop, 1, staggered_reset=True,
                              hint_engines=hint):
                    sweep()
            else:
                sweep()
    nc.compile()
    return nc


def _make_in_maps(x: np.ndarray, scale: int):
    xf = np.asarray(x, dtype=np.float32).reshape(-1, C)
    W = _bf16(_build_w(scale))
    return [{"x": _shard_xt(xf, i), "w": W} for i in range(N_CORES)]


def kernel(x: np.ndarray, scale) -> np.ndarray:
    import sys
    if "/opt/trn_rl_repo" not in sys.path:
        sys.path.insert(0, "/opt/trn_rl_repo")
    from concourse.bass_utils import run_bass_kernel_spmd

    scale = int(np.asarray(scale))
    x = np.asarray(x, dtype=np.float32)
    orig_shape = x.shape

    if "nc" not in _CACHE:
        _CACHE["nc"] = _build_nc()
    nc = _CACHE["nc"]

    in_maps = _make_in_maps(x, scale)
    res = run_bass_kernel_spmd(nc, in_maps, list(range(N_CORES)))
    outs = []
    for i, r in enumerate(res.results):
        yc = np.asarray(r["y"], dtype=np.float32)
        yc = yc.reshape(128, TILES, C).transpose(1, 0, 2).reshape(-1, C)
        outs.append(yc[:_TILE_CNT[i] * 128])
    y = np.concatenate(outs, axis=0).reshape(orig_shape)
    return y.astype(np.float32)
